# revision 3
# baseline (speedup 1.0000x reference)
"""CKGGCN GNN message-passing kernel for 8 TRN2 NeuronCores (Bass/Tile).

Strategy (see spec sharding_hint): edges are sharded by DESTINATION range
(head entity / user / item) so every segment-sum is tile-local and done with
one-hot matmuls accumulating in PSUM; embedding tables are rebuilt per layer
and replicated to every core with an AllGather. Per-edge source rows are
fetched with dma_gather (int16-windowed, 4 SWDGE queues). All one-hot
matrices are precomputed on the host (index-space only) and streamed in as
bf16, so the vector engine only does the actual per-edge math.

Per layer, per core:
  KG pass:    score = <Q[head], Q[tail]*rel>, w = exp(score/sqrt(32)),
              U[j] += w*(E[tail]*rel), S[j] += w  (PSUM, one-hot matmuls)
              agg = normalize(U/S)
  IU pass:    user_agg[u] += w_e * E[item]
  IE pass:    E_next = agg + sum w_e * U[user];  Q_next = E_next @ W_Q
  AllGather packed E|Q and U tables for the next layer.
Outputs are the mean over layer snapshots, assembled on the host.
"""
import sys

sys.path.insert(0, "/opt/trn_rl_repo")
import math
import os

import numpy as np
import ml_dtypes

import concourse.bass as bass
import concourse.bacc as bacc
import concourse.mybir as mybir
import concourse.tile as tile
from concourse.bass_utils import run_bass_kernel_spmd
from concourse.masks import make_identity

BF16 = mybir.dt.bfloat16
F32 = mybir.dt.float32
I16 = mybir.dt.int16
P = 128
NCORES = 8
NQ = 4  # SWDGE queues
MAX_CHUNKS_PER_GATHER = 8  # num_idxs <= 1024
BF = ml_dtypes.bfloat16


class Cfg:
    def __init__(self, n_ent, n_usr, d, nrel, n_layers, win_cap=25088):
        assert d == 64
        self.n_ent, self.n_usr, self.d, self.nrel = n_ent, n_usr, d, nrel
        self.n_layers = n_layers
        self.ent_slice = -(-n_ent // NCORES)
        self.ent_pad = -(-self.ent_slice // P) * P
        self.te = self.ent_pad // P
        self.eq_rows = NCORES * self.ent_pad
        self.usr_slice = -(-n_usr // NCORES)
        self.usr_pad = -(-self.usr_slice // P) * P
        self.tu = self.usr_pad // P
        self.u_rows = NCORES * self.usr_pad
        # gather windows (int16 row index limit)
        self.ent_nw = max(1, -(-self.eq_rows // win_cap))
        self.ent_wr = -(-self.eq_rows // self.ent_nw)
        assert self.ent_wr <= 32767
        self.usr_nw = max(1, -(-self.u_rows // win_cap))
        self.usr_wr = -(-self.u_rows // self.usr_nw)
        assert self.usr_wr <= 32767

    def pk_ent(self, n):
        return self.ent_pad * (n // self.ent_slice) + (n % self.ent_slice)

    def pk_usr(self, u):
        return self.usr_pad * (u // self.usr_slice) + (u % self.usr_slice)


def _pack_idx(flat):
    """Pack a flat [128*B] row-index array into the dma_gather int16 layout
    [128, 8*B], replicated across all eight 16-partition groups."""
    n = len(flat)
    assert n % P == 0
    ncol = n // 16
    s = np.arange(ncol)
    pp = np.arange(P)
    jm = (s[None, :] // 8) * P + (s[None, :] % 8) * 16 + (pp[:, None] % 16)
    return flat[jm].astype(np.int16)


class PassStreams:
    """Host-built streams + shared program metadata for one edge pass."""

    def __init__(self, cfg, n_tiles, nw, dst_local, win, winrow, payload, kind):
        """dst_local/win/winrow/payload: per-core lists of per-edge arrays.
        kind: 'kg' (payload=relation type) or 'w' (payload=edge weight)."""
        self.kind = kind
        tl = [d // P for d in dst_local]
        jl = [d % P for d in dst_local]
        counts = np.zeros((NCORES, n_tiles, nw), np.int64)
        orders = []
        for c in range(NCORES):
            key = tl[c] * nw + win[c]
            o = np.lexsort((jl[c], key))
            orders.append(o)
            np.add.at(counts[c].reshape(-1), key, 1)
        nchunks = -(-counts // P)  # per-core chunks per (t,w)
        B = nchunks.max(axis=0)  # shared across cores [n_tiles, nw]
        # units: (t, w, chunk_off, nchunk<=8) lists; chunk global numbering
        units = []  # (t, w, g0, bg)
        tile_units = [[] for _ in range(n_tiles)]
        g = 0
        for t in range(n_tiles):
            for w in range(nw):
                b = int(B[t, w])
                while b > 0:
                    bg = min(b, MAX_CHUNKS_PER_GATHER)
                    tile_units[t].append((len(units), t, w, g, bg))
                    units.append((t, w, g, bg))
                    g += bg
                    b -= bg
        self.total_chunks = g
        self.units = units
        self.tile_units = tile_units
        self.B = B
        # slot/column offsets per (t,w): chunk offsets
        chunk_off = np.zeros((n_tiles, nw), np.int64)
        acc = 0
        for t in range(n_tiles):
            for w in range(nw):
                chunk_off[t, w] = acc
                acc += int(B[t, w])
        assert acc == g

        # build streams per core
        gcols = g * P  # one-hot stream columns
        self.idx = np.zeros((NCORES, P, g * 8), np.int16)
        self.M = np.zeros((NCORES, P, gcols), BF)
        if kind == "kg":
            self.MT = np.zeros((NCORES, P, gcols), BF)
            self.RT = np.zeros((NCORES, 16, gcols), BF)
        else:
            self.W = np.zeros((NCORES, P, g), BF)
        for c in range(NCORES):
            o = orders[c]
            tl_s, jl_s = tl[c][o], jl[c][o]
            win_s, wr_s, pay_s = win[c][o], winrow[c][o], payload[c][o]
            key_s = tl_s * nw + win_s
            cnt_flat = counts[c].reshape(-1)
            starts = np.zeros_like(cnt_flat)
            starts[1:] = np.cumsum(cnt_flat)[:-1]
            slot = np.arange(len(o)) - starts[key_s]
            gchunk = chunk_off[tl_s, win_s] + slot // P  # global chunk id
            pslot = slot % P
            self.M[c][pslot, gchunk * P + jl_s] = 1.0
            if kind == "kg":
                self.MT[c][jl_s, gchunk * P + pslot] = 1.0
                self.RT[c][pay_s, gchunk * P + pslot] = 1.0
            else:
                self.W[c][pslot, gchunk] = pay_s.astype(BF)
            # gather index packing: per unit
            flatrows = np.zeros(g * P, np.int64)
            flatrows[gchunk * P + pslot] = wr_s
            for (t, w, g0, bg) in units:
                sub = flatrows[g0 * P:(g0 + bg) * P]
                self.idx[c][:, g0 * 8:(g0 + bg) * 8] = _pack_idx(sub)


def host_prep(cfg, edge_index, edge_type, inter_edge, inter_edge_w):
    head = edge_index[0].astype(np.int64)
    tail = edge_index[1].astype(np.int64)
    ty = edge_type.astype(np.int64) - 1
    usr = inter_edge[0].astype(np.int64)
    itm = inter_edge[1].astype(np.int64)
    w = inter_edge_w.astype(np.float32)

    def split(core_of, arrs):
        out = [[] for _ in arrs]
        for c in range(NCORES):
            m = core_of == c
            for i, a in enumerate(arrs):
                out[i].append(a[m])
        return out

    # KG: shard by head range, gather packed-entity rows of tail
    c_kg = head // cfg.ent_slice
    hl, tr, tyl = split(c_kg, [head % cfg.ent_slice, cfg.pk_ent(tail), ty])
    kg = PassStreams(cfg, cfg.te, cfg.ent_nw,
                     hl, [r // cfg.ent_wr for r in tr],
                     [r % cfg.ent_wr for r in tr], tyl, "kg")
    # IU: shard by user range, gather packed-entity rows of item
    c_iu = usr // cfg.usr_slice
    ul, ir, wl = split(c_iu, [usr % cfg.usr_slice, cfg.pk_ent(itm), w])
    iu = PassStreams(cfg, cfg.tu, cfg.ent_nw,
                     ul, [r // cfg.ent_wr for r in ir],
                     [r % cfg.ent_wr for r in ir], wl, "w")
    # IE: shard by item range, gather packed-user rows of user
    c_ie = itm // cfg.ent_slice
    il, ur, wl2 = split(c_ie, [itm % cfg.ent_slice, cfg.pk_usr(usr), w])
    ie = PassStreams(cfg, cfg.te, cfg.usr_nw,
                     il, [r // cfg.usr_wr for r in ur],
                     [r % cfg.usr_wr for r in ur], wl2, "w")
    return kg, iu, ie


def build_program(cfg, kg, iu, ie):
    nc = bacc.Bacc("TRN2", target_bir_lowering=False, num_devices=NCORES,
                   num_swdge_queues=NQ)
    D = cfg.d
    L = cfg.n_layers

    # ---------------- I/O ----------------
    e0 = nc.dram_tensor("e0", [cfg.ent_pad, D], F32, kind="ExternalInput")
    u0 = nc.dram_tensor("u0", [cfg.usr_pad, D], F32, kind="ExternalInput")
    wq_in = nc.dram_tensor("wq", [D, D], BF16, kind="ExternalInput")
    rel_in = nc.dram_tensor("rel", [16, D], BF16, kind="ExternalInput")
    kg_idx = nc.dram_tensor("kg_idx", [P, kg.total_chunks * 8], I16, kind="ExternalInput")
    kg_M = nc.dram_tensor("kg_M", [P, kg.total_chunks * P], BF16, kind="ExternalInput")
    kg_MT = nc.dram_tensor("kg_MT", [P, kg.total_chunks * P], BF16, kind="ExternalInput")
    kg_RT = nc.dram_tensor("kg_RT", [16, kg.total_chunks * P], BF16, kind="ExternalInput")
    iu_idx = nc.dram_tensor("iu_idx", [P, iu.total_chunks * 8], I16, kind="ExternalInput")
    iu_M = nc.dram_tensor("iu_M", [P, iu.total_chunks * P], BF16, kind="ExternalInput")
    iu_W = nc.dram_tensor("iu_W", [P, iu.total_chunks], BF16, kind="ExternalInput")
    ie_idx = nc.dram_tensor("ie_idx", [P, ie.total_chunks * 8], I16, kind="ExternalInput")
    ie_M = nc.dram_tensor("ie_M", [P, ie.total_chunks * P], BF16, kind="ExternalInput")
    ie_W = nc.dram_tensor("ie_W", [P, ie.total_chunks], BF16, kind="ExternalInput")
    e_out = nc.dram_tensor("e_out", [cfg.ent_pad, D], F32, kind="ExternalOutput")
    u_out = nc.dram_tensor("u_out", [cfg.usr_pad, D], F32, kind="ExternalOutput")

    # ---------------- internals ----------------
    eq_slice = [nc.dram_tensor(f"eq_slice{i}", [cfg.ent_pad, 2 * D], BF16) for i in range(2)]
    u_slice = [nc.dram_tensor(f"u_slice{i}", [cfg.usr_pad, 2 * D], BF16) for i in range(2)]
    eq_full = [nc.dram_tensor(f"eq_full{i}", [cfg.eq_rows, 2 * D], BF16, addr_space="Shared")
               for i in range(2)]
    u_full = [nc.dram_tensor(f"u_full{i}", [cfg.u_rows, 2 * D], BF16, addr_space="Shared")
              for i in range(2)]
    aggn_d = nc.dram_tensor("aggn", [cfg.ent_pad, D], F32)
    eouts = [nc.dram_tensor(f"eout{l}", [cfg.ent_pad, D], F32) for l in range(L)]
    uouts = [nc.dram_tensor(f"uout{l}", [cfg.usr_pad, D], F32) for l in range(L)]

    groups = [list(range(NCORES))]
    qrr = [0]  # round-robin queue counter

    with tile.TileContext(nc) as tc:
        with (
            tc.tile_pool(name="const", bufs=1) as cp,
            tc.tile_pool(name="stage", bufs=3) as stp,
            tc.tile_pool(name="gath", bufs=3) as gp,
            tc.tile_pool(name="onehot", bufs=3) as mp,
            tc.tile_pool(name="work", bufs=3) as sp,
            tc.tile_pool(name="small", bufs=4) as smp,
            tc.tile_pool(name="psA", bufs=2, space="PSUM") as psA,
            tc.tile_pool(name="psB", bufs=2, space="PSUM") as psB,
            tc.tile_pool(name="psAcc", bufs=2, space="PSUM") as psAcc,
        ):
            ident = cp.tile([P, P], BF16)
            make_identity(nc, ident[:])
            wq_sb = cp.tile([D, D], BF16)
            nc.sync.dma_start(out=wq_sb[:], in_=wq_in[:, :])
            rel_sb = cp.tile([16, D], BF16)
            nc.sync.dma_start(out=rel_sb[:], in_=rel_in[:, :])

            def eq_tile_build(src_f32_rows, dst_rows, lbl):
                """E rows (f32 DRAM) -> packed [E|Q] bf16 rows in dst."""
                st = stp.tile([P, 2 * D], BF16, tag="eqstage")
                ef = smp.tile([P, D], F32, tag="ef32")
                nc.sync.dma_start(out=ef[:], in_=src_f32_rows)
                nc.vector.tensor_copy(out=st[:, 0:D], in_=ef[:])
                tp = psA.tile([D, P], BF16, tag="a")
                nc.tensor.transpose(out=tp[:], in_=st[:, 0:D], identity=ident[:])
                et = smp.tile([D, P], BF16, tag="ett")
                nc.scalar.copy(out=et[:], in_=tp[:])
                qp = psB.tile([P, D], F32, tag="b")
                nc.tensor.matmul(out=qp[:], lhsT=et[:], rhs=wq_sb[:], start=True, stop=True)
                nc.scalar.copy(out=st[:, D:2 * D], in_=qp[:])
                nc.sync.dma_start(out=dst_rows, in_=st[:])

            # ---------------- prep: layer-0 tables ----------------
            with nc.named_scope("prep"):
                for t in range(cfg.te):
                    eq_tile_build(e0[t * P:(t + 1) * P, :], eq_slice[0][t * P:(t + 1) * P, :], f"p{t}")
                for t in range(cfg.tu):
                    st = stp.tile([P, 2 * D], BF16, tag="eqstage")
                    uf = smp.tile([P, D], F32, tag="ef32")
                    nc.sync.dma_start(out=uf[:], in_=u0[t * P:(t + 1) * P, :])
                    nc.vector.tensor_copy(out=st[:, 0:D], in_=uf[:])
                    nc.vector.memzero(st[:, D:2 * D])
                    nc.sync.dma_start(out=u_slice[0][t * P:(t + 1) * P, :], in_=st[:])
            with nc.named_scope("ag0"):
                nc.gpsimd.collective_compute(
                    "AllGather", mybir.AluOpType.bypass, groups,
                    ins=[eq_slice[0][:, :]], outs=[eq_full[0][:, :]])
                nc.gpsimd.collective_compute(
                    "AllGather", mybir.AluOpType.bypass, groups,
                    ins=[u_slice[0][:, :]], outs=[u_full[0][:, :]])

            def do_gather(ps, unit, idx_t, full_t, wr, nw):
                """Issue one dma_gather for a pass unit; returns [128, bg, 128] bf16 tile."""
                (uid, t, w, g0, bg) = unit
                ni = bg * P
                ix = smp.tile([P, 8 * bg], I16, tag="ix")
                nc.sync.dma_start(out=ix[:], in_=idx_t[:, g0 * 8:(g0 + bg) * 8])
                g = gp.tile([P, bg, 2 * D], BF16, tag="g")
                lo = w * wr
                hi = min(lo + wr, full_t.shape[0])
                nc.gpsimd.dma_gather(g[:], full_t[lo:hi, :], ix[:], ni, ni, 2 * D,
                                     queue_num=qrr[0] % NQ)
                qrr[0] += 1
                return g

            def kg_tile(t, eqf, lbl):
                units = kg.tile_units[t]
                if not units:
                    z = sp.tile([P, D], F32, tag="aggn")
                    nc.vector.memzero(z[:])
                    nc.sync.dma_start(out=aggn_d[t * P:(t + 1) * P, :], in_=z[:])
                    return
                nunits = len(units)
                qtile = smp.tile([P, D], BF16, tag="qtile")
                nc.sync.dma_start(out=qtile[:], in_=eq_slice[0 if lbl == 0 else 1][t * P:(t + 1) * P, D:2 * D])
                us = psAcc.tile([P, 72], F32, tag="acc")  # U 0:64, S 64:66
                first = True
                for ui, unit in enumerate(units):
                    (uid, _t, w, g0, bg) = unit
                    gt = do_gather(None, unit, kg_idx, eqf, cfg.ent_wr, cfg.ent_nw)
                    mt = mp.tile([P, bg * P], BF16, tag="m")
                    nc.sync.dma_start(out=mt[:], in_=kg_M[:, g0 * P:(g0 + bg) * P])
                    mtt = mp.tile([P, bg * P], BF16, tag="mt")
                    nc.sync.dma_start(out=mtt[:], in_=kg_MT[:, g0 * P:(g0 + bg) * P])
                    rtt = mp.tile([16, bg * P], BF16, tag="rt")
                    nc.sync.dma_start(out=rtt[:], in_=kg_RT[:, g0 * P:(g0 + bg) * P])
                    relx_ps = psA.tile([P, bg * D], F32, tag="a")
                    qh_ps = psB.tile([P, bg * D], F32, tag="b")
                    for k in range(bg):
                        nc.tensor.matmul(out=relx_ps[:, k * D:(k + 1) * D],
                                         lhsT=rtt[:, k * P:(k + 1) * P], rhs=rel_sb[:],
                                         start=True, stop=True)
                        nc.tensor.matmul(out=qh_ps[:, k * D:(k + 1) * D],
                                         lhsT=mtt[:, k * P:(k + 1) * P], rhs=qtile[:],
                                         start=True, stop=True)
                    relx = sp.tile([P, bg * D], BF16, tag="relxs")
                    nc.scalar.copy(out=relx[:], in_=relx_ps[:])
                    qh = sp.tile([P, bg * D], BF16, tag="qhs")
                    nc.scalar.copy(out=qh[:], in_=qh_ps[:])
                    g3 = gt[:]  # [P, bg, 128]
                    g1 = sp.tile([P, bg * D], BF16, tag="g1")
                    nc.vector.tensor_mul(out=g1[:].rearrange("p (b d) -> p b d", d=D),
                                         in0=g3[:, :, D:2 * D], in1=relx[:].rearrange("p (b d) -> p b d", d=D))
                    x = sp.tile([P, bg * D], BF16, tag="x")
                    nc.vector.tensor_mul(out=x[:], in0=g1[:], in1=qh[:])
                    sc = smp.tile([P, bg * 2], F32, tag="sc")
                    nc.vector.reduce_sum(out=sc[:], in_=x[:].rearrange("p (h d) -> p h d", d=32),
                                         axis=mybir.AxisListType.X)
                    wm = sp.tile([P, bg * 66], BF16, tag="wm")
                    wm3 = wm[:].rearrange("p (b c) -> p b c", c=66)
                    nc.scalar.activation(out=wm3[:, :, D:66],
                                         in_=sc[:].rearrange("p (b h) -> p b h", h=2),
                                         func=mybir.ActivationFunctionType.Exp,
                                         scale=float(1.0 / math.sqrt(32.0)))
                    v1 = sp.tile([P, bg * D], BF16, tag="v1")
                    nc.vector.tensor_mul(out=v1[:].rearrange("p (b d) -> p b d", d=D),
                                         in0=g3[:, :, 0:D], in1=relx[:].rearrange("p (b d) -> p b d", d=D))
                    wb = wm3[:, :, D:66]
                    nc.vector.tensor_mul(
                        out=wm3[:, :, 0:D].rearrange("p b (h d) -> p b h d", d=32),
                        in0=v1[:].rearrange("p (b h d) -> p b h d", h=2, d=32),
                        in1=wb[:, :, :, None].to_broadcast([P, bg, 2, 32]))
                    for k in range(bg):
                        nc.tensor.matmul(out=us[:, 0:66],
                                         lhsT=mt[:, k * P:(k + 1) * P],
                                         rhs=wm[:, k * 66:(k + 1) * 66],
                                         start=first,
                                         stop=(ui == nunits - 1 and k == bg - 1),
                                         skip_group_check=True)
                        first = False
                # finalize tile t -> aggn
                sm = smp.tile([P, 2], F32, tag="sm")
                nc.vector.tensor_scalar_max(out=sm[:], in0=us[:, D:66], scalar1=1e-30)
                srec = smp.tile([P, 2], F32, tag="srec")
                nc.vector.reciprocal(out=srec[:], in_=sm[:])
                agg = sp.tile([P, D], F32, tag="agg")
                nc.vector.tensor_mul(out=agg[:].rearrange("p (h d) -> p h d", d=32),
                                     in0=us[:, 0:D].rearrange("p (h d) -> p h d", d=32),
                                     in1=srec[:][:, :, None].to_broadcast([P, 2, 32]))
                sq = sp.tile([P, D], F32, tag="sq")
                ssum = smp.tile([P, 1], F32, tag="ssum")
                nc.scalar.activation(out=sq[:], in_=agg[:],
                                     func=mybir.ActivationFunctionType.Square,
                                     accum_out=ssum[:])
                nrm = smp.tile([P, 1], F32, tag="nrm")
                nc.scalar.sqrt(out=nrm[:], in_=ssum[:])
                nc.vector.tensor_scalar_max(out=nrm[:], in0=nrm[:], scalar1=1e-12)
                rn = smp.tile([P, 1], F32, tag="rn")
                nc.vector.reciprocal(out=rn[:], in_=nrm[:])
                aggn = sp.tile([P, D], F32, tag="aggn")
                nc.vector.tensor_mul(out=aggn[:], in0=agg[:],
                                     in1=rn[:].to_broadcast([P, D]))
                nc.sync.dma_start(out=aggn_d[t * P:(t + 1) * P, :], in_=aggn[:])

            def inter_tile(t, strm, idx_t, M_t, W_t, full_t, wr, nw, bufname):
                units = strm.tile_units[t]
                acc = psAcc.tile([P, 72], F32, tag="acc")
                first = True
                nunits = len(units)
                for ui, unit in enumerate(units):
                    (uid, _t, w, g0, bg) = unit
                    gt = do_gather(None, unit, idx_t, full_t, wr, nw)
                    mt = mp.tile([P, bg * P], BF16, tag="m")
                    nc.sync.dma_start(out=mt[:], in_=M_t[:, g0 * P:(g0 + bg) * P])
                    wt = smp.tile([P, bg], BF16, tag="wt")
                    nc.sync.dma_start(out=wt[:], in_=W_t[:, g0:g0 + bg])
                    msg = sp.tile([P, bg * D], BF16, tag="msg")
                    nc.vector.tensor_mul(
                        out=msg[:].rearrange("p (b d) -> p b d", d=D),
                        in0=gt[:, :, 0:D],
                        in1=wt[:][:, :, None].to_broadcast([P, bg, D]))
                    for k in range(bg):
                        nc.tensor.matmul(out=acc[:, 0:D],
                                         lhsT=mt[:, k * P:(k + 1) * P],
                                         rhs=msg[:, k * D:(k + 1) * D],
                                         start=first,
                                         stop=(ui == nunits - 1 and k == bg - 1),
                                         skip_group_check=True)
                        first = False
                if first:  # no edges at all for this tile
                    z = sp.tile([P, D], F32, tag="zz")
                    nc.vector.memzero(z[:])
                    return z
                out = sp.tile([P, D], F32, tag="iaccs")
                nc.vector.tensor_copy(out=out[:], in_=acc[:, 0:D])
                return out

            # ---------------- layers ----------------
            for l in range(L):
                cur, nxt = l % 2, (l + 1) % 2
                eqf, uf = eq_full[cur], u_full[cur]
                last = l == L - 1
                with nc.named_scope(f"kg{l}"):
                    for t in range(cfg.te):
                        kg_tile(t, eqf, l)
                with nc.named_scope(f"iu{l}"):
                    for t in range(cfg.tu):
                        ua = inter_tile(t, iu, iu_idx, iu_M, iu_W, eqf,
                                        cfg.ent_wr, cfg.ent_nw, "iu")
                        nc.sync.dma_start(out=uouts[l][t * P:(t + 1) * P, :], in_=ua[:])
                        if not last:
                            st = stp.tile([P, 2 * D], BF16, tag="eqstage")
                            nc.vector.tensor_copy(out=st[:, 0:D], in_=ua[:])
                            nc.vector.memzero(st[:, D:2 * D])
                            nc.sync.dma_start(out=u_slice[1][t * P:(t + 1) * P, :], in_=st[:])
                with nc.named_scope(f"ie{l}"):
                    for t in range(cfg.te):
                        p2 = inter_tile(t, ie, ie_idx, ie_M, ie_W, uf,
                                        cfg.usr_wr, cfg.usr_nw, "ie")
                        an = sp.tile([P, D], F32, tag="an")
                        nc.sync.dma_start(out=an[:], in_=aggn_d[t * P:(t + 1) * P, :])
                        en = sp.tile([P, D], F32, tag="en")
                        nc.vector.tensor_add(out=en[:], in0=an[:], in1=p2[:])
                        nc.sync.dma_start(out=eouts[l][t * P:(t + 1) * P, :], in_=en[:])
                        if not last:
                            st = stp.tile([P, 2 * D], BF16, tag="eqstage")
                            nc.vector.tensor_copy(out=st[:, 0:D], in_=en[:])
                            tp = psA.tile([D, P], BF16, tag="a")
                            nc.tensor.transpose(out=tp[:], in_=st[:, 0:D], identity=ident[:])
                            et = smp.tile([D, P], BF16, tag="ett")
                            nc.scalar.copy(out=et[:], in_=tp[:])
                            qp = psB.tile([P, D], F32, tag="b")
                            nc.tensor.matmul(out=qp[:], lhsT=et[:], rhs=wq_sb[:],
                                             start=True, stop=True)
                            nc.scalar.copy(out=st[:, D:2 * D], in_=qp[:])
                            nc.sync.dma_start(out=eq_slice[1][t * P:(t + 1) * P, :], in_=st[:])
                if not last:
                    with nc.named_scope(f"ag{l + 1}"):
                        nc.gpsimd.collective_compute(
                            "AllGather", mybir.AluOpType.bypass, groups,
                            ins=[eq_slice[1][:, :]], outs=[eq_full[nxt][:, :]])
                        nc.gpsimd.collective_compute(
                            "AllGather", mybir.AluOpType.bypass, groups,
                            ins=[u_slice[1][:, :]], outs=[u_full[nxt][:, :]])

            # ---------------- outputs: mean over layers ----------------
            with nc.named_scope("out"):
                inv = float(1.0 / (L + 1))
                for t in range(cfg.te):
                    a = sp.tile([P, D], F32, tag="oa")
                    nc.sync.dma_start(out=a[:], in_=e0[t * P:(t + 1) * P, :])
                    for l in range(L):
                        b = sp.tile([P, D], F32, tag="ob")
                        nc.sync.dma_start(out=b[:], in_=eouts[l][t * P:(t + 1) * P, :])
                        nc.vector.tensor_add(out=a[:], in0=a[:], in1=b[:])
                    nc.scalar.mul(out=a[:], in_=a[:], mul=inv)
                    nc.sync.dma_start(out=e_out[t * P:(t + 1) * P, :], in_=a[:])
                for t in range(cfg.tu):
                    a = sp.tile([P, D], F32, tag="oa")
                    nc.sync.dma_start(out=a[:], in_=u0[t * P:(t + 1) * P, :])
                    for l in range(L):
                        b = sp.tile([P, D], F32, tag="ob")
                        nc.sync.dma_start(out=b[:], in_=uouts[l][t * P:(t + 1) * P, :])
                        nc.vector.tensor_add(out=a[:], in0=a[:], in1=b[:])
                    nc.scalar.mul(out=a[:], in_=a[:], mul=inv)
                    nc.sync.dma_start(out=u_out[t * P:(t + 1) * P, :], in_=a[:])

    nc.compile()
    return nc


def _pad_rows(a, rows):
    out = np.zeros((rows, a.shape[1]), a.dtype)
    out[:a.shape[0]] = a
    return out


def prepare(layers_num, user_emb, entity_emb, inter_edge, inter_edge_w,
            edge_index, edge_type, relation_emb, W_Q, win_cap=25088):
    """Build (cfg, nc, in_maps) for the given full inputs."""
    L = int(np.asarray(layers_num))
    user_emb = np.asarray(user_emb, np.float32)
    entity_emb = np.asarray(entity_emb, np.float32)
    inter_edge = np.asarray(inter_edge)
    inter_edge_w = np.asarray(inter_edge_w, np.float32)
    edge_index = np.asarray(edge_index)
    edge_type = np.asarray(edge_type)
    relation_emb = np.asarray(relation_emb, np.float32)
    W_Q = np.asarray(W_Q, np.float32)

    n_usr, d = user_emb.shape
    n_ent = entity_emb.shape[0]
    nrel = relation_emb.shape[0]
    cfg = Cfg(n_ent, n_usr, d, nrel, L, win_cap=win_cap)
    kg, iu, ie = host_prep(cfg, edge_index, edge_type, inter_edge, inter_edge_w)
    nc = build_program(cfg, kg, iu, ie)

    rel16 = np.zeros((16, d), BF)
    rel16[:nrel] = relation_emb.astype(BF)
    in_maps = []
    for c in range(NCORES):
        es = _pad_rows(entity_emb[c * cfg.ent_slice:(c + 1) * cfg.ent_slice], cfg.ent_pad)
        us = _pad_rows(user_emb[c * cfg.usr_slice:(c + 1) * cfg.usr_slice], cfg.usr_pad)
        in_maps.append({
            "e0": es, "u0": us,
            "wq": W_Q.astype(BF), "rel": rel16,
            "kg_idx": kg.idx[c], "kg_M": kg.M[c], "kg_MT": kg.MT[c], "kg_RT": kg.RT[c],
            "iu_idx": iu.idx[c], "iu_M": iu.M[c], "iu_W": iu.W[c],
            "ie_idx": ie.idx[c], "ie_M": ie.M[c], "ie_W": ie.W[c],
        })
    return cfg, nc, in_maps


def assemble(cfg, per_core_outs, n_usr, n_ent):
    u_parts = [per_core_outs[c]["u_out"][:cfg.usr_slice] for c in range(NCORES)]
    e_parts = [per_core_outs[c]["e_out"][:cfg.ent_slice] for c in range(NCORES)]
    user_out = np.concatenate(u_parts, axis=0)[:n_usr]
    entity_out = np.concatenate(e_parts, axis=0)[:n_ent]
    return (np.ascontiguousarray(user_out), np.ascontiguousarray(entity_out))


def kernel(layers_num, user_emb, entity_emb, inter_edge, inter_edge_w,
           edge_index, edge_type, relation_emb, W_Q, _trace=False):
    n_usr = user_emb.shape[0]
    n_ent = entity_emb.shape[0]
    cfg, nc, in_maps = prepare(layers_num, user_emb, entity_emb, inter_edge,
                               inter_edge_w, edge_index, edge_type,
                               relation_emb, W_Q)
    res = run_bass_kernel_spmd(nc, in_maps, core_ids=list(range(NCORES)), trace=_trace)
    kernel.last_result = res
    return assemble(cfg, res.results, n_usr, n_ent)


# revision 4
# speedup vs baseline: 1.0948x; 1.0948x over previous
"""CKGGCN GNN message-passing kernel for 8 TRN2 NeuronCores (Bass/Tile).

Strategy (see spec sharding_hint): edges are sharded by DESTINATION range
(head entity / user / item) so every segment-sum is tile-local and done with
one-hot matmuls accumulating in PSUM; embedding tables are rebuilt per layer
and replicated to every core with an AllGather. Per-edge source rows are
fetched with dma_gather (int16-windowed, 4 SWDGE queues). All one-hot
matrices are precomputed on the host (index-space only) and streamed in as
bf16, so the vector engine only does the actual per-edge math.

Per layer, per core:
  KG pass:    score = <Q[head], Q[tail]*rel>, w = exp(score/sqrt(32)),
              U[j] += w*(E[tail]*rel), S[j] += w  (PSUM, one-hot matmuls)
              agg = normalize(U/S)
  IU pass:    user_agg[u] += w_e * E[item]
  IE pass:    E_next = agg + sum w_e * U[user];  Q_next = E_next @ W_Q
  AllGather packed E|Q and U tables for the next layer.
Outputs are the mean over layer snapshots, assembled on the host.
"""
import sys

sys.path.insert(0, "/opt/trn_rl_repo")
import math
import os

import numpy as np
import ml_dtypes

import concourse.bass as bass
import concourse.bacc as bacc
import concourse.mybir as mybir
import concourse.tile as tile
from concourse.bass_utils import run_bass_kernel_spmd
from concourse.masks import make_identity

BF16 = mybir.dt.bfloat16
F32 = mybir.dt.float32
I16 = mybir.dt.int16
P = 128
NCORES = 8
NQ = 4  # SWDGE queues
MAX_CHUNKS_PER_GATHER = 8  # num_idxs <= 1024
BF = ml_dtypes.bfloat16


class Cfg:
    def __init__(self, n_ent, n_usr, d, nrel, n_layers, win_cap=25088):
        assert d == 64
        self.n_ent, self.n_usr, self.d, self.nrel = n_ent, n_usr, d, nrel
        self.n_layers = n_layers
        self.ent_slice = -(-n_ent // NCORES)
        self.ent_pad = -(-self.ent_slice // P) * P
        self.te = self.ent_pad // P
        self.eq_rows = NCORES * self.ent_pad
        self.usr_slice = -(-n_usr // NCORES)
        self.usr_pad = -(-self.usr_slice // P) * P
        self.tu = self.usr_pad // P
        self.u_rows = NCORES * self.usr_pad
        # gather windows (int16 row index limit)
        self.ent_nw = max(1, -(-self.eq_rows // win_cap))
        self.ent_wr = -(-self.eq_rows // self.ent_nw)
        assert self.ent_wr <= 32767
        self.usr_nw = max(1, -(-self.u_rows // win_cap))
        self.usr_wr = -(-self.u_rows // self.usr_nw)
        assert self.usr_wr <= 32767

    def pk_ent(self, n):
        return self.ent_pad * (n // self.ent_slice) + (n % self.ent_slice)

    def pk_usr(self, u):
        return self.usr_pad * (u // self.usr_slice) + (u % self.usr_slice)


def _pack_idx(flat):
    """Pack a flat [128*B] row-index array into the dma_gather int16 layout
    [128, 8*B], replicated across all eight 16-partition groups."""
    n = len(flat)
    assert n % P == 0
    ncol = n // 16
    s = np.arange(ncol)
    pp = np.arange(P)
    jm = (s[None, :] // 8) * P + (s[None, :] % 8) * 16 + (pp[:, None] % 16)
    return flat[jm].astype(np.int16)


class PassStreams:
    """Host-built streams + shared program metadata for one edge pass."""

    def __init__(self, cfg, n_tiles, nw, dst_local, win, winrow, payload, kind):
        """dst_local/win/winrow/payload: per-core lists of per-edge arrays.
        kind: 'kg' (payload=relation type) or 'w' (payload=edge weight)."""
        self.kind = kind
        tl = [d // P for d in dst_local]
        jl = [d % P for d in dst_local]
        counts = np.zeros((NCORES, n_tiles, nw), np.int64)
        orders = []
        for c in range(NCORES):
            key = tl[c] * nw + win[c]
            o = np.lexsort((jl[c], key))
            orders.append(o)
            np.add.at(counts[c].reshape(-1), key, 1)
        nchunks = -(-counts // P)  # per-core chunks per (t,w)
        B = nchunks.max(axis=0)  # shared across cores [n_tiles, nw]
        # units: (t, w, chunk_off, nchunk<=8) lists; chunk global numbering
        units = []  # (t, w, g0, bg)
        tile_units = [[] for _ in range(n_tiles)]
        g = 0
        for t in range(n_tiles):
            for w in range(nw):
                b = int(B[t, w])
                while b > 0:
                    bg = min(b, MAX_CHUNKS_PER_GATHER)
                    tile_units[t].append((len(units), t, w, g, bg))
                    units.append((t, w, g, bg))
                    g += bg
                    b -= bg
        self.total_chunks = g
        self.units = units
        self.tile_units = tile_units
        self.B = B
        # slot/column offsets per (t,w): chunk offsets
        chunk_off = np.zeros((n_tiles, nw), np.int64)
        acc = 0
        for t in range(n_tiles):
            for w in range(nw):
                chunk_off[t, w] = acc
                acc += int(B[t, w])
        assert acc == g

        # build streams per core
        gcols = g * P  # one-hot stream columns
        self.idx = np.zeros((NCORES, P, g * 8), np.int16)
        self.M = np.zeros((NCORES, P, gcols), BF)
        if kind == "kg":
            self.MT = np.zeros((NCORES, P, gcols), BF)
            self.RT = np.zeros((NCORES, 16, gcols), BF)
        else:
            self.W = np.zeros((NCORES, P, g), BF)
        for c in range(NCORES):
            o = orders[c]
            tl_s, jl_s = tl[c][o], jl[c][o]
            win_s, wr_s, pay_s = win[c][o], winrow[c][o], payload[c][o]
            key_s = tl_s * nw + win_s
            cnt_flat = counts[c].reshape(-1)
            starts = np.zeros_like(cnt_flat)
            starts[1:] = np.cumsum(cnt_flat)[:-1]
            slot = np.arange(len(o)) - starts[key_s]
            gchunk = chunk_off[tl_s, win_s] + slot // P  # global chunk id
            pslot = slot % P
            self.M[c][pslot, gchunk * P + jl_s] = 1.0
            if kind == "kg":
                self.MT[c][jl_s, gchunk * P + pslot] = 1.0
                self.RT[c][pay_s, gchunk * P + pslot] = 1.0
            else:
                self.W[c][pslot, gchunk] = pay_s.astype(BF)
            # gather index packing: per unit
            flatrows = np.zeros(g * P, np.int64)
            flatrows[gchunk * P + pslot] = wr_s
            for (t, w, g0, bg) in units:
                sub = flatrows[g0 * P:(g0 + bg) * P]
                self.idx[c][:, g0 * 8:(g0 + bg) * 8] = _pack_idx(sub)


def host_prep(cfg, edge_index, edge_type, inter_edge, inter_edge_w):
    head = edge_index[0].astype(np.int64)
    tail = edge_index[1].astype(np.int64)
    ty = edge_type.astype(np.int64) - 1
    usr = inter_edge[0].astype(np.int64)
    itm = inter_edge[1].astype(np.int64)
    w = inter_edge_w.astype(np.float32)

    def split(core_of, arrs):
        out = [[] for _ in arrs]
        for c in range(NCORES):
            m = core_of == c
            for i, a in enumerate(arrs):
                out[i].append(a[m])
        return out

    # KG: shard by head range, gather packed-entity rows of tail
    c_kg = head // cfg.ent_slice
    hl, tr, tyl = split(c_kg, [head % cfg.ent_slice, cfg.pk_ent(tail), ty])
    kg = PassStreams(cfg, cfg.te, cfg.ent_nw,
                     hl, [r // cfg.ent_wr for r in tr],
                     [r % cfg.ent_wr for r in tr], tyl, "kg")
    # IU: shard by user range, gather packed-entity rows of item
    c_iu = usr // cfg.usr_slice
    ul, ir, wl = split(c_iu, [usr % cfg.usr_slice, cfg.pk_ent(itm), w])
    iu = PassStreams(cfg, cfg.tu, cfg.ent_nw,
                     ul, [r // cfg.ent_wr for r in ir],
                     [r % cfg.ent_wr for r in ir], wl, "w")
    # IE: shard by item range, gather packed-user rows of user
    c_ie = itm // cfg.ent_slice
    il, ur, wl2 = split(c_ie, [itm % cfg.ent_slice, cfg.pk_usr(usr), w])
    ie = PassStreams(cfg, cfg.te, cfg.usr_nw,
                     il, [r // cfg.usr_wr for r in ur],
                     [r % cfg.usr_wr for r in ur], wl2, "w")
    return kg, iu, ie


def build_program(cfg, kg, iu, ie):
    nc = bacc.Bacc("TRN2", target_bir_lowering=False, num_devices=NCORES,
                   num_swdge_queues=NQ)
    D = cfg.d
    L = cfg.n_layers

    # ---------------- I/O ----------------
    e0 = nc.dram_tensor("e0", [cfg.ent_pad, D], F32, kind="ExternalInput")
    u0 = nc.dram_tensor("u0", [cfg.usr_pad, D], F32, kind="ExternalInput")
    wq_in = nc.dram_tensor("wq", [D, D], BF16, kind="ExternalInput")
    rel_in = nc.dram_tensor("rel", [16, D], BF16, kind="ExternalInput")
    kg_idx = nc.dram_tensor("kg_idx", [P, kg.total_chunks * 8], I16, kind="ExternalInput")
    kg_M = nc.dram_tensor("kg_M", [P, kg.total_chunks * P], BF16, kind="ExternalInput")
    kg_MT = nc.dram_tensor("kg_MT", [P, kg.total_chunks * P], BF16, kind="ExternalInput")
    kg_RT = nc.dram_tensor("kg_RT", [16, kg.total_chunks * P], BF16, kind="ExternalInput")
    iu_idx = nc.dram_tensor("iu_idx", [P, iu.total_chunks * 8], I16, kind="ExternalInput")
    iu_M = nc.dram_tensor("iu_M", [P, iu.total_chunks * P], BF16, kind="ExternalInput")
    iu_W = nc.dram_tensor("iu_W", [P, iu.total_chunks], BF16, kind="ExternalInput")
    ie_idx = nc.dram_tensor("ie_idx", [P, ie.total_chunks * 8], I16, kind="ExternalInput")
    ie_M = nc.dram_tensor("ie_M", [P, ie.total_chunks * P], BF16, kind="ExternalInput")
    ie_W = nc.dram_tensor("ie_W", [P, ie.total_chunks], BF16, kind="ExternalInput")
    e_out = nc.dram_tensor("e_out", [cfg.ent_pad, D], F32, kind="ExternalOutput")
    u_out = nc.dram_tensor("u_out", [cfg.usr_pad, D], F32, kind="ExternalOutput")

    # ---------------- internals ----------------
    eq_slice = [nc.dram_tensor(f"eq_slice{i}", [cfg.ent_pad, 2 * D], BF16) for i in range(2)]
    u_slice = [nc.dram_tensor(f"u_slice{i}", [cfg.usr_pad, 2 * D], BF16) for i in range(2)]
    eq_full = [nc.dram_tensor(f"eq_full{i}", [cfg.eq_rows, 2 * D], BF16, addr_space="Shared")
               for i in range(2)]
    u_full = [nc.dram_tensor(f"u_full{i}", [cfg.u_rows, 2 * D], BF16, addr_space="Shared")
              for i in range(2)]
    aggn_d = nc.dram_tensor("aggn", [cfg.ent_pad, D], F32)
    eouts = [nc.dram_tensor(f"eout{l}", [cfg.ent_pad, D], F32) for l in range(L)]
    uouts = [nc.dram_tensor(f"uout{l}", [cfg.usr_pad, D], F32) for l in range(L)]

    groups = [list(range(NCORES))]
    qrr = [0]  # round-robin queue counter

    with tile.TileContext(nc) as tc:
        with (
            tc.tile_pool(name="const", bufs=1) as cp,
            tc.tile_pool(name="stage", bufs=3) as stp,
            tc.tile_pool(name="gath", bufs=8) as gp,
            tc.tile_pool(name="onehot", bufs=6) as mp,
            tc.tile_pool(name="work", bufs=5) as sp,
            tc.tile_pool(name="small", bufs=10) as smp,
            tc.tile_pool(name="psA", bufs=2, space="PSUM") as psA,
            tc.tile_pool(name="psB", bufs=2, space="PSUM") as psB,
            tc.tile_pool(name="psAcc", bufs=2, space="PSUM") as psAcc,
        ):
            ident = cp.tile([P, P], BF16)
            make_identity(nc, ident[:])
            wq_sb = cp.tile([D, D], BF16)
            nc.sync.dma_start(out=wq_sb[:], in_=wq_in[:, :])
            rel_sb = cp.tile([16, D], BF16)
            nc.sync.dma_start(out=rel_sb[:], in_=rel_in[:, :])

            def eq_tile_build(src_f32_rows, dst_rows, lbl):
                """E rows (f32 DRAM) -> packed [E|Q] bf16 rows in dst."""
                st = stp.tile([P, 2 * D], BF16, tag="eqstage")
                ef = smp.tile([P, D], F32, tag="ef32")
                nc.sync.dma_start(out=ef[:], in_=src_f32_rows)
                nc.vector.tensor_copy(out=st[:, 0:D], in_=ef[:])
                tp = psA.tile([D, P], BF16, tag="a")
                nc.tensor.transpose(out=tp[:], in_=st[:, 0:D], identity=ident[:])
                et = smp.tile([D, P], BF16, tag="ett")
                nc.scalar.copy(out=et[:], in_=tp[:])
                qp = psB.tile([P, D], F32, tag="b")
                nc.tensor.matmul(out=qp[:], lhsT=et[:], rhs=wq_sb[:], start=True, stop=True)
                nc.scalar.copy(out=st[:, D:2 * D], in_=qp[:])
                nc.sync.dma_start(out=dst_rows, in_=st[:])

            # ---------------- prep: layer-0 tables ----------------
            with nc.named_scope("prep"):
                for t in range(cfg.te):
                    eq_tile_build(e0[t * P:(t + 1) * P, :], eq_slice[0][t * P:(t + 1) * P, :], f"p{t}")
                for t in range(cfg.tu):
                    st = stp.tile([P, 2 * D], BF16, tag="eqstage")
                    uf = smp.tile([P, D], F32, tag="ef32")
                    nc.sync.dma_start(out=uf[:], in_=u0[t * P:(t + 1) * P, :])
                    nc.vector.tensor_copy(out=st[:, 0:D], in_=uf[:])
                    nc.vector.memzero(st[:, D:2 * D])
                    nc.sync.dma_start(out=u_slice[0][t * P:(t + 1) * P, :], in_=st[:])
            with nc.named_scope("ag0"):
                nc.gpsimd.collective_compute(
                    "AllGather", mybir.AluOpType.bypass, groups,
                    ins=[eq_slice[0][:, :]], outs=[eq_full[0][:, :]])
                nc.gpsimd.collective_compute(
                    "AllGather", mybir.AluOpType.bypass, groups,
                    ins=[u_slice[0][:, :]], outs=[u_full[0][:, :]])

            def do_gather(ps, unit, idx_t, full_t, wr, nw):
                """Issue one dma_gather for a pass unit; returns [128, bg, 128] bf16 tile."""
                (uid, t, w, g0, bg) = unit
                ni = bg * P
                ix = smp.tile([P, 8 * bg], I16, tag="ix")
                nc.sync.dma_start(out=ix[:], in_=idx_t[:, g0 * 8:(g0 + bg) * 8])
                g = gp.tile([P, bg, 2 * D], BF16, tag="g")
                lo = w * wr
                hi = min(lo + wr, full_t.shape[0])
                nc.gpsimd.dma_gather(g[:], full_t[lo:hi, :], ix[:], ni, ni, 2 * D,
                                     queue_num=qrr[0] % NQ)
                qrr[0] += 1
                return g

            def kg_tile(t, eqf, lbl):
                units = kg.tile_units[t]
                if not units:
                    z = sp.tile([P, D], F32, tag="aggn")
                    nc.vector.memzero(z[:])
                    nc.sync.dma_start(out=aggn_d[t * P:(t + 1) * P, :], in_=z[:])
                    return
                nunits = len(units)
                qtile = smp.tile([P, D], BF16, tag="qtile")
                nc.sync.dma_start(out=qtile[:], in_=eq_slice[0 if lbl == 0 else 1][t * P:(t + 1) * P, D:2 * D])
                us = psAcc.tile([P, 72], F32, tag="acc")  # U 0:64, S 64:66
                first = True
                for ui, unit in enumerate(units):
                    (uid, _t, w, g0, bg) = unit
                    gt = do_gather(None, unit, kg_idx, eqf, cfg.ent_wr, cfg.ent_nw)
                    mt = mp.tile([P, bg * P], BF16, tag="m")
                    nc.sync.dma_start(out=mt[:], in_=kg_M[:, g0 * P:(g0 + bg) * P])
                    mtt = mp.tile([P, bg * P], BF16, tag="mt")
                    nc.sync.dma_start(out=mtt[:], in_=kg_MT[:, g0 * P:(g0 + bg) * P])
                    rtt = mp.tile([16, bg * P], BF16, tag="rt")
                    nc.sync.dma_start(out=rtt[:], in_=kg_RT[:, g0 * P:(g0 + bg) * P])
                    relx_ps = psA.tile([P, bg * D], F32, tag="a")
                    qh_ps = psB.tile([P, bg * D], F32, tag="b")
                    for k in range(bg):
                        nc.tensor.matmul(out=relx_ps[:, k * D:(k + 1) * D],
                                         lhsT=rtt[:, k * P:(k + 1) * P], rhs=rel_sb[:],
                                         start=True, stop=True)
                        nc.tensor.matmul(out=qh_ps[:, k * D:(k + 1) * D],
                                         lhsT=mtt[:, k * P:(k + 1) * P], rhs=qtile[:],
                                         start=True, stop=True)
                    relx = sp.tile([P, bg * D], BF16, tag="relxs")
                    nc.scalar.copy(out=relx[:], in_=relx_ps[:])
                    qh = sp.tile([P, bg * D], BF16, tag="qhs")
                    nc.scalar.copy(out=qh[:], in_=qh_ps[:])
                    g3 = gt[:]  # [P, bg, 128]
                    g1 = sp.tile([P, bg * D], BF16, tag="g1")
                    nc.vector.tensor_mul(out=g1[:].rearrange("p (b d) -> p b d", d=D),
                                         in0=g3[:, :, D:2 * D], in1=relx[:].rearrange("p (b d) -> p b d", d=D))
                    x = sp.tile([P, bg * D], BF16, tag="x")
                    nc.vector.tensor_mul(out=x[:], in0=g1[:], in1=qh[:])
                    sc = smp.tile([P, bg * 2], F32, tag="sc")
                    nc.vector.reduce_sum(out=sc[:], in_=x[:].rearrange("p (h d) -> p h d", d=32),
                                         axis=mybir.AxisListType.X)
                    wm = sp.tile([P, bg * 66], BF16, tag="wm")
                    wm3 = wm[:].rearrange("p (b c) -> p b c", c=66)
                    nc.scalar.activation(out=wm3[:, :, D:66],
                                         in_=sc[:].rearrange("p (b h) -> p b h", h=2),
                                         func=mybir.ActivationFunctionType.Exp,
                                         scale=float(1.0 / math.sqrt(32.0)))
                    v1 = sp.tile([P, bg * D], BF16, tag="v1")
                    nc.vector.tensor_mul(out=v1[:].rearrange("p (b d) -> p b d", d=D),
                                         in0=g3[:, :, 0:D], in1=relx[:].rearrange("p (b d) -> p b d", d=D))
                    wb = wm3[:, :, D:66]
                    nc.vector.tensor_mul(
                        out=wm3[:, :, 0:D].rearrange("p b (h d) -> p b h d", d=32),
                        in0=v1[:].rearrange("p (b h d) -> p b h d", h=2, d=32),
                        in1=wb[:, :, :, None].to_broadcast([P, bg, 2, 32]))
                    for k in range(bg):
                        nc.tensor.matmul(out=us[:, 0:66],
                                         lhsT=mt[:, k * P:(k + 1) * P],
                                         rhs=wm[:, k * 66:(k + 1) * 66],
                                         start=first,
                                         stop=(ui == nunits - 1 and k == bg - 1),
                                         skip_group_check=True)
                        first = False
                # finalize tile t -> aggn
                sm = smp.tile([P, 2], F32, tag="sm")
                nc.vector.tensor_scalar_max(out=sm[:], in0=us[:, D:66], scalar1=1e-30)
                srec = smp.tile([P, 2], F32, tag="srec")
                nc.vector.reciprocal(out=srec[:], in_=sm[:])
                agg = sp.tile([P, D], F32, tag="agg")
                nc.vector.tensor_mul(out=agg[:].rearrange("p (h d) -> p h d", d=32),
                                     in0=us[:, 0:D].rearrange("p (h d) -> p h d", d=32),
                                     in1=srec[:][:, :, None].to_broadcast([P, 2, 32]))
                sq = sp.tile([P, D], F32, tag="sq")
                ssum = smp.tile([P, 1], F32, tag="ssum")
                nc.scalar.activation(out=sq[:], in_=agg[:],
                                     func=mybir.ActivationFunctionType.Square,
                                     accum_out=ssum[:])
                nrm = smp.tile([P, 1], F32, tag="nrm")
                nc.scalar.sqrt(out=nrm[:], in_=ssum[:])
                nc.vector.tensor_scalar_max(out=nrm[:], in0=nrm[:], scalar1=1e-12)
                rn = smp.tile([P, 1], F32, tag="rn")
                nc.vector.reciprocal(out=rn[:], in_=nrm[:])
                aggn = sp.tile([P, D], F32, tag="aggn")
                nc.vector.tensor_mul(out=aggn[:], in0=agg[:],
                                     in1=rn[:].to_broadcast([P, D]))
                nc.sync.dma_start(out=aggn_d[t * P:(t + 1) * P, :], in_=aggn[:])

            def inter_tile(t, strm, idx_t, M_t, W_t, full_t, wr, nw, bufname):
                units = strm.tile_units[t]
                acc = psAcc.tile([P, 72], F32, tag="acc")
                first = True
                nunits = len(units)
                for ui, unit in enumerate(units):
                    (uid, _t, w, g0, bg) = unit
                    gt = do_gather(None, unit, idx_t, full_t, wr, nw)
                    mt = mp.tile([P, bg * P], BF16, tag="m")
                    nc.sync.dma_start(out=mt[:], in_=M_t[:, g0 * P:(g0 + bg) * P])
                    wt = smp.tile([P, bg], BF16, tag="wt")
                    nc.sync.dma_start(out=wt[:], in_=W_t[:, g0:g0 + bg])
                    msg = sp.tile([P, bg * D], BF16, tag="msg")
                    nc.vector.tensor_mul(
                        out=msg[:].rearrange("p (b d) -> p b d", d=D),
                        in0=gt[:, :, 0:D],
                        in1=wt[:][:, :, None].to_broadcast([P, bg, D]))
                    for k in range(bg):
                        nc.tensor.matmul(out=acc[:, 0:D],
                                         lhsT=mt[:, k * P:(k + 1) * P],
                                         rhs=msg[:, k * D:(k + 1) * D],
                                         start=first,
                                         stop=(ui == nunits - 1 and k == bg - 1),
                                         skip_group_check=True)
                        first = False
                if first:  # no edges at all for this tile
                    z = sp.tile([P, D], F32, tag="zz")
                    nc.vector.memzero(z[:])
                    return z
                out = sp.tile([P, D], F32, tag="iaccs")
                nc.vector.tensor_copy(out=out[:], in_=acc[:, 0:D])
                return out

            # ---------------- layers ----------------
            for l in range(L):
                cur, nxt = l % 2, (l + 1) % 2
                eqf, uf = eq_full[cur], u_full[cur]
                last = l == L - 1
                with nc.named_scope(f"kg{l}"):
                    for t in range(cfg.te):
                        kg_tile(t, eqf, l)
                with nc.named_scope(f"iu{l}"):
                    for t in range(cfg.tu):
                        ua = inter_tile(t, iu, iu_idx, iu_M, iu_W, eqf,
                                        cfg.ent_wr, cfg.ent_nw, "iu")
                        nc.sync.dma_start(out=uouts[l][t * P:(t + 1) * P, :], in_=ua[:])
                        if not last:
                            st = stp.tile([P, 2 * D], BF16, tag="eqstage")
                            nc.vector.tensor_copy(out=st[:, 0:D], in_=ua[:])
                            nc.vector.memzero(st[:, D:2 * D])
                            nc.sync.dma_start(out=u_slice[1][t * P:(t + 1) * P, :], in_=st[:])
                with nc.named_scope(f"ie{l}"):
                    for t in range(cfg.te):
                        p2 = inter_tile(t, ie, ie_idx, ie_M, ie_W, uf,
                                        cfg.usr_wr, cfg.usr_nw, "ie")
                        an = sp.tile([P, D], F32, tag="an")
                        nc.sync.dma_start(out=an[:], in_=aggn_d[t * P:(t + 1) * P, :])
                        en = sp.tile([P, D], F32, tag="en")
                        nc.vector.tensor_add(out=en[:], in0=an[:], in1=p2[:])
                        nc.sync.dma_start(out=eouts[l][t * P:(t + 1) * P, :], in_=en[:])
                        if not last:
                            st = stp.tile([P, 2 * D], BF16, tag="eqstage")
                            nc.vector.tensor_copy(out=st[:, 0:D], in_=en[:])
                            tp = psA.tile([D, P], BF16, tag="a")
                            nc.tensor.transpose(out=tp[:], in_=st[:, 0:D], identity=ident[:])
                            et = smp.tile([D, P], BF16, tag="ett")
                            nc.scalar.copy(out=et[:], in_=tp[:])
                            qp = psB.tile([P, D], F32, tag="b")
                            nc.tensor.matmul(out=qp[:], lhsT=et[:], rhs=wq_sb[:],
                                             start=True, stop=True)
                            nc.scalar.copy(out=st[:, D:2 * D], in_=qp[:])
                            nc.sync.dma_start(out=eq_slice[1][t * P:(t + 1) * P, :], in_=st[:])
                if not last:
                    with nc.named_scope(f"ag{l + 1}"):
                        nc.gpsimd.collective_compute(
                            "AllGather", mybir.AluOpType.bypass, groups,
                            ins=[eq_slice[1][:, :]], outs=[eq_full[nxt][:, :]])
                        nc.gpsimd.collective_compute(
                            "AllGather", mybir.AluOpType.bypass, groups,
                            ins=[u_slice[1][:, :]], outs=[u_full[nxt][:, :]])

            # ---------------- outputs: mean over layers ----------------
            with nc.named_scope("out"):
                inv = float(1.0 / (L + 1))
                for t in range(cfg.te):
                    a = sp.tile([P, D], F32, tag="oa")
                    nc.sync.dma_start(out=a[:], in_=e0[t * P:(t + 1) * P, :])
                    for l in range(L):
                        b = sp.tile([P, D], F32, tag="ob")
                        nc.sync.dma_start(out=b[:], in_=eouts[l][t * P:(t + 1) * P, :])
                        nc.vector.tensor_add(out=a[:], in0=a[:], in1=b[:])
                    nc.scalar.mul(out=a[:], in_=a[:], mul=inv)
                    nc.sync.dma_start(out=e_out[t * P:(t + 1) * P, :], in_=a[:])
                for t in range(cfg.tu):
                    a = sp.tile([P, D], F32, tag="oa")
                    nc.sync.dma_start(out=a[:], in_=u0[t * P:(t + 1) * P, :])
                    for l in range(L):
                        b = sp.tile([P, D], F32, tag="ob")
                        nc.sync.dma_start(out=b[:], in_=uouts[l][t * P:(t + 1) * P, :])
                        nc.vector.tensor_add(out=a[:], in0=a[:], in1=b[:])
                    nc.scalar.mul(out=a[:], in_=a[:], mul=inv)
                    nc.sync.dma_start(out=u_out[t * P:(t + 1) * P, :], in_=a[:])

    nc.compile()
    return nc


def _pad_rows(a, rows):
    out = np.zeros((rows, a.shape[1]), a.dtype)
    out[:a.shape[0]] = a
    return out


def prepare(layers_num, user_emb, entity_emb, inter_edge, inter_edge_w,
            edge_index, edge_type, relation_emb, W_Q, win_cap=25088):
    """Build (cfg, nc, in_maps) for the given full inputs."""
    L = int(np.asarray(layers_num))
    user_emb = np.asarray(user_emb, np.float32)
    entity_emb = np.asarray(entity_emb, np.float32)
    inter_edge = np.asarray(inter_edge)
    inter_edge_w = np.asarray(inter_edge_w, np.float32)
    edge_index = np.asarray(edge_index)
    edge_type = np.asarray(edge_type)
    relation_emb = np.asarray(relation_emb, np.float32)
    W_Q = np.asarray(W_Q, np.float32)

    n_usr, d = user_emb.shape
    n_ent = entity_emb.shape[0]
    nrel = relation_emb.shape[0]
    cfg = Cfg(n_ent, n_usr, d, nrel, L, win_cap=win_cap)
    kg, iu, ie = host_prep(cfg, edge_index, edge_type, inter_edge, inter_edge_w)
    nc = build_program(cfg, kg, iu, ie)

    rel16 = np.zeros((16, d), BF)
    rel16[:nrel] = relation_emb.astype(BF)
    in_maps = []
    for c in range(NCORES):
        es = _pad_rows(entity_emb[c * cfg.ent_slice:(c + 1) * cfg.ent_slice], cfg.ent_pad)
        us = _pad_rows(user_emb[c * cfg.usr_slice:(c + 1) * cfg.usr_slice], cfg.usr_pad)
        in_maps.append({
            "e0": es, "u0": us,
            "wq": W_Q.astype(BF), "rel": rel16,
            "kg_idx": kg.idx[c], "kg_M": kg.M[c], "kg_MT": kg.MT[c], "kg_RT": kg.RT[c],
            "iu_idx": iu.idx[c], "iu_M": iu.M[c], "iu_W": iu.W[c],
            "ie_idx": ie.idx[c], "ie_M": ie.M[c], "ie_W": ie.W[c],
        })
    return cfg, nc, in_maps


def assemble(cfg, per_core_outs, n_usr, n_ent):
    u_parts = [per_core_outs[c]["u_out"][:cfg.usr_slice] for c in range(NCORES)]
    e_parts = [per_core_outs[c]["e_out"][:cfg.ent_slice] for c in range(NCORES)]
    user_out = np.concatenate(u_parts, axis=0)[:n_usr]
    entity_out = np.concatenate(e_parts, axis=0)[:n_ent]
    return (np.ascontiguousarray(user_out), np.ascontiguousarray(entity_out))


def kernel(layers_num, user_emb, entity_emb, inter_edge, inter_edge_w,
           edge_index, edge_type, relation_emb, W_Q, _trace=False):
    n_usr = user_emb.shape[0]
    n_ent = entity_emb.shape[0]
    cfg, nc, in_maps = prepare(layers_num, user_emb, entity_emb, inter_edge,
                               inter_edge_w, edge_index, edge_type,
                               relation_emb, W_Q)
    res = run_bass_kernel_spmd(nc, in_maps, core_ids=list(range(NCORES)), trace=_trace)
    kernel.last_result = res
    return assemble(cfg, res.results, n_usr, n_ent)
